# revision 50
# baseline (speedup 1.0000x reference)
"""Trainium2 Bass kernel for nn_AttentionHead (B=8, S=2048, E=1024, Dh=64).

Sharding: data-parallel over batch B across the 8 NeuronCores (one batch
element per core); W/b replicated; results gathered on host.

Per-core computation in transposed orientation (scores^T[k,q]):
  qkv = x @ W + b   (f32r; 1/sqrt(Dh) folded into W_q,b_q; the k-bias is
                     dropped entirely - it only adds a per-query constant
                     to scores, which softmax over k is invariant to up to
                     the masked entries, ~1e-3 error)
  scores^T = kT.T @ qT                  (PE f32r, 2-bank PSUM pair)
  u = exp(scores^T)                     (ACT, one instr per k-chunk pair,
                                         PSUM -> SBUF bf16)
  u = where(mask^T, 1, u)               (DVE copy_predicated; mask^T lives
                                         only in PSUM via bf16-bitcast PE
                                         transposes, re-viewed as int16)
  [num^T; Z] = [v | 1]^T @ u            (PE bf16 accumulate over k-chunks)
  out = (num * (1/Z))^T                 (numz copy, PE transpose, DVE recip,
                                         DVE/ACT per-partition scale)

DMA: transfer time occupies the issuing engine queue in the cost model
(~332 B/ns per queue) but different queues overlap. Bulk streams ride the
two compute-free queues:
  SP (sync):    x "a" half-blocks (block 0 quartered for an early PE
                start), then even mask tiles, last block's output store
  Pool (SWDGE): x "b" half-blocks, then odd mask tiles, output stores
  ACT:          only the small early W/bias loads
With DMA parallelized the kernel is PE-bound: ~66 us PE busy out of a
~79 us span (x-transposes f32r 10.2, qkv 13.7, scores 13.7,
mask-transposes 13.7, av 13.7; contraction=64 halves the array for
scores/av - unavoidable at Dh=64). DVE ~61 (copy_predicated 40 is
DVE-only and pinned), ACT ~60 (exp 41 pinned). A ~1.3 us dummy-matmul
warmup ramps the PE p-state before the first x tiles land. fp8/DoubleRow
was tried for qkv and av and REJECTED: per-element e4m3 quantization noise
(~4-6%) survives dot products at full relative strength (measured 9e-2
rel err vs the 2e-2 gate).

Cost-model span ~79.1 us/core (baseline 85.6); max rel err vs fp32
reference ~5.5e-3 (bf16 u/v rounding, dropped k-bias; f32r elsewhere).
"""

import os
import sys

sys.path.insert(0, "/opt/trn_rl_repo")

import numpy as np

import concourse.bass as bass
import concourse.tile as tile
from concourse import bacc, mybir
from concourse.masks import make_identity
from concourse.bass_utils import run_bass_kernel_spmd

F32 = mybir.dt.float32
F32R = mybir.dt.float32r
BF16 = mybir.dt.bfloat16
I32 = mybir.dt.int32
I16 = mybir.dt.int16

B, S, E, DH = 8, 2048, 1024, 64
N_CORES = 8
SCALE = 1.0 / 8.0  # 1/sqrt(DH)

MM_DT = F32R

EC = E // 128     # 8   e-chunks
SC = S // 128     # 16  s-chunks (k-chunks)
QB = S // 512     # 4   x s-blocks
KC = SC

BLOCKS = [(0, 512), (512, 512), (1024, 512), (1536, 256), (1792, 256)]

AF = mybir.ActivationFunctionType
OP = mybir.AluOpType

# bulk DMA queues: SP (no compute) and Pool (nearly idle)
MASK_ENG = ["sync", "gpsimd"]


def build(nc: bass.Bass):
    x_in = nc.dram_tensor("x", [S, E], F32, kind="ExternalInput")
    m_in = nc.dram_tensor("mask", [S, S], I32, kind="ExternalInput")
    w_in = nc.dram_tensor("W", [E, 3 * DH], F32, kind="ExternalInput")
    b_in = nc.dram_tensor("b", [3 * DH], F32, kind="ExternalInput")
    o_out = nc.dram_tensor("out", [S, DH], F32, kind="ExternalOutput")

    trace_sim = bool(os.environ.get("TRN_TRACE_SIM"))
    with tile.TileContext(nc, trace_sim=trace_sim) as tc:
        with (
            tc.tile_pool(name="persist", bufs=1) as persist,
            tc.tile_pool(name="small", bufs=1) as small,
        ):
            # ---- constants / weights (Pool queue) ------------------------
            ident = persist.tile([128, 128], F32)
            make_identity(nc, ident)
            ident_bf = persist.tile([128, 128], BF16)
            nc.vector.tensor_copy(ident_bf[:], ident[:])
            ident_r = persist.tile([128, 128], F32R)
            nc.vector.tensor_copy(ident_r[:], ident[:])
            ones_bf = persist.tile([128, 2, 512], BF16)
            nc.gpsimd.memset(ones_bf[:], 1.0)

            # PE clock warmup: ~3us of dummy matmuls from t~0.2 so the PE
            # p-state is fully ramped when the first real transposes arrive
            warm_w = small.tile([128, 128], BF16)
            nc.vector.memset(warm_w[:], 0.0)
            with tc.tile_pool(name="ps_w", bufs=1, space="PSUM") as ps_w:
                ps_warm = ps_w.tile([128, 128], F32, name="psw")
                for _ in range(12):
                    nc.tensor.matmul(
                        ps_warm[:], warm_w[:], warm_w[:], start=True, stop=True
                    )

            w_raw = small.tile([128, EC, 3 * DH], F32)
            nc.scalar.dma_start(w_raw[:], w_in.rearrange("(o p) d -> p o d", p=128))
            # stacked f32r stationaries: wst1 = [W_q*scale | W_k],
            # wst2 = [W_k | W_v]  (fp8 would be ~5-9% error: per-element
            # quantization noise survives the contraction at full relative
            # strength)
            wst1 = persist.tile([128, EC, 128], F32R)
            wst2 = persist.tile([128, EC, 128], F32R)
            nc.vector.tensor_scalar_mul(wst1[:, :, 0:64], w_raw[:, :, 0:DH], SCALE)
            nc.scalar.copy(wst1[:, :, 64:128], w_raw[:, :, DH : 2 * DH])
            nc.vector.tensor_copy(wst2[:, :, 0:64], w_raw[:, :, DH : 2 * DH])
            nc.scalar.copy(wst2[:, :, 64:128], w_raw[:, :, 2 * DH : 3 * DH])

            # b_k is dropped: a k-bias adds a per-query constant to scores,
            # which softmax over k is invariant to (the bq.k0 cross-term is
            # kept via b_q). kT is then a pure psum copy.
            b_q_raw = small.tile([64, 1], F32)
            nc.scalar.dma_start(b_q_raw[:], b_in[0:64].unsqueeze(-1))
            b_q = persist.tile([64, 1], F32)
            nc.vector.tensor_scalar_mul(b_q[:], b_q_raw[:], SCALE)
            b_v128 = persist.tile([128, 1], F32)  # v bias on lanes 64-127
            nc.scalar.dma_start(b_v128[64:128, :], b_in[128:192].unsqueeze(-1))

            # warm the ACT exp table early
            warm = small.tile([1, 1], F32)
            nc.vector.memset(warm[:], 0.0)
            warm_o = small.tile([1, 1], BF16)
            nc.scalar.activation(warm_o[:], warm[:], AF.Exp)

            # persistent activations: qv rows 0-63 = q^T, rows 64-127 = v^T
            qv = persist.tile([128, S], MM_DT)
            kT = persist.tile([64, S], MM_DT)
            v1 = persist.tile([128, SC, DH + 1], BF16)  # v natural + ones col
            nc.gpsimd.memset(v1[:, :, DH : DH + 1], 1.0)

            from contextlib import ExitStack

            mask_ctx = ExitStack()
            # 9 rotating slots: tile n+9 reuses tile n's space, whose consumer
            # block finishes well before tile n+9 is needed (no deadlock: the
            # attend consumes tiles strictly in order)
            p_m = mask_ctx.enter_context(tc.tile_pool(name="mstage", bufs=9))

            # ---- phase 1: x -> x^T -> q/k/v ------------------------------
            with (
                tc.tile_pool(name="xnat", bufs=2) as p_xnat,
                tc.tile_pool(name="xT", bufs=2) as p_xT,
                tc.tile_pool(name="ps_t", bufs=4, space="PSUM") as ps_t,
                tc.tile_pool(name="ps_mm", bufs=4, space="PSUM") as ps_mm,
            ):
                x_nats = []
                for nt in range(QB):
                    # x half-blocks: "a" half on SP, "b" half on Pool; the
                    # first block is quartered so transposes start early
                    x_nat4 = p_xnat.tile([128, 4, E], F32R)
                    if nt == 0:
                        for q, eng in enumerate((nc.sync, nc.gpsimd) * 2):
                            eng.dma_start(
                                x_nat4[:, q : q + 1, :],
                                x_in[q * 128 : (q + 1) * 128, :]
                                .rearrange("(c p) e -> p c e", p=128)
                                .bitcast(F32R),
                            )
                    else:
                        for h, eng in ((0, nc.sync), (1, nc.gpsimd)):
                            eng.dma_start(
                                x_nat4[:, h * 2 : (h + 1) * 2, :],
                                x_in[nt * 512 + h * 256 : nt * 512 + (h + 1) * 256, :]
                                .rearrange("(c p) e -> p c e", p=128)
                                .bitcast(F32R),
                            )
                    x_nats.append(x_nat4)

                # mask tile DMAs, round-robin over the three queues, emitted
                # after x so each queue drains x first (program order)
                m_tiles = {}
                for mc in range(SC):
                    m_i32 = p_m.tile([128, S], I32)
                    eng = getattr(nc, MASK_ENG[mc % 2])
                    eng.dma_start(m_i32[:], m_in[mc * 128 : (mc + 1) * 128, :])
                    m_tiles[mc] = m_i32

                for nt in range(QB):
                    x_nat4 = x_nats[nt]
                    x_T = p_xT.tile([128, EC, 512], F32R)
                    # all transposes+copies of the block first, qkv after:
                    # keeps the in-order PE queue free of qkv instructions
                    # that would stall waiting on the copies
                    for c4 in range(4):
                        for g in range(2):
                            pst = ps_t.tile([128, 512], F32R, name="pst")
                            for j4 in range(4):
                                j = g * 4 + j4
                                nc.tensor.transpose(
                                    pst[:, j4 * 128 : (j4 + 1) * 128],
                                    x_nat4[:, c4, j * 128 : (j + 1) * 128],
                                    ident_r[:],
                                )
                            dst = x_T[
                                :, g * 4 : (g + 1) * 4,
                                c4 * 128 : (c4 + 1) * 128,
                            ]
                            src = pst[:].rearrange("p (j f) -> p j f", j=4)
                            if (c4 + g) % 2 == 0:
                                nc.vector.tensor_copy(dst, src)
                            else:
                                nc.scalar.copy(dst, src)  # ACT

                    for p2 in range(2):
                        ps1 = ps_mm.tile([128, 256], F32, name="psmm")
                        ps2 = ps_mm.tile([128, 256], F32, name="psmm")
                        cs = slice(p2 * 256, (p2 + 1) * 256)
                        gsl = slice(nt * 512 + p2 * 256, nt * 512 + (p2 + 1) * 256)
                        for j in range(EC):
                            nc.tensor.matmul(
                                ps1[:], wst1[:, j, :], x_T[:, j, cs],
                                start=(j == 0), stop=(j == EC - 1),
                            )
                        for j in range(EC):
                            nc.tensor.matmul(
                                ps2[:], wst2[:, j, :], x_T[:, j, cs],
                                start=(j == 0), stop=(j == EC - 1),
                            )
                        # kT is a pure copy (k-bias dropped: softmax-invariant)
                        nc.vector.tensor_scalar_add(
                            qv[0:64, gsl], ps1[0:64, :], b_q[:]
                        )
                        nc.scalar.copy(kT[:, gsl], ps2[0:64, :])
                        nc.vector.tensor_scalar_add(
                            qv[64:128, gsl], ps2[64:128, :], b_v128[64:128, :]
                        )

                    # v natural (+ones col) for this block's four s-chunks
                    psv = ps_mm.tile([128, 256], F32R, name="psmm")
                    for j in range(4):
                        c = nt * 4 + j
                        nc.tensor.transpose(
                            psv[:, j * 64 : (j + 1) * 64],
                            qv[64:128, c * 128 : (c + 1) * 128],
                            ident_r[64:128, 64:128],
                        )
                    nc.scalar.copy(
                        v1[:, nt * 4 : (nt + 1) * 4, 0:DH],
                        psv[:, 0:256].rearrange("p (j f) -> p j f", j=4),
                    )

            # ---- phase 2: attention --------------------------------------
            with mask_ctx:
                with (
                    tc.tile_pool(name="u", bufs=6) as p_u,
                    tc.tile_pool(name="nz", bufs=3) as p_nz,
                    tc.tile_pool(name="osb", bufs=3) as p_o,
                    tc.tile_pool(name="ps_s", bufs=2, space="PSUM") as ps_s,
                    tc.tile_pool(name="ps_tm", bufs=2, space="PSUM") as ps_tm,
                    tc.tile_pool(name="ps_o", bufs=1, space="PSUM") as ps_o,
                    tc.tile_pool(name="ps_t2", bufs=1, space="PSUM") as ps_t2,
                ):
                    for bi, (q0, width) in enumerate(BLOCKS):
                        nmc = width // 128
                        mviews = [
                            m_tiles[q0 // 128 + mc][:].bitcast(BF16)
                            for mc in range(nmc)
                        ]
                        po_f = ps_o.tile([DH + 1, 512], F32, name="po")
                        po = po_f[:, :width]
                        # software-pipelined: transposes+scores of pair p
                        # are emitted before exp/cp/av of pair p-1
                        stage = []
                        for kp in range(KC // 2):
                            pstm_f = ps_tm.tile([128, 2, 512], BF16, name="pstm")
                            upair_f = p_u.tile([128, 2, 512], BF16, tag="u")
                            pss_f = ps_s.tile([128, 2, 512], F32, name="pss")
                            for j in range(2):
                                kc = kp * 2 + j
                                pstm = pstm_f[:, j, :width]
                                for mc in range(nmc):
                                    nc.tensor.matmul(
                                        pstm[:, mc * 128 : (mc + 1) * 128],
                                        mviews[mc][:, 2 * kc * 128 : 2 * (kc + 1) * 128 : 2],
                                        ident_bf[:],
                                        is_transpose=True,
                                    )
                                nc.tensor.matmul(
                                    pss_f[:, j, :width],
                                    kT[:, kc * 128 : (kc + 1) * 128],
                                    qv[0:64, q0 : q0 + width],
                                    start=True,
                                    stop=True,
                                )
                            stage.append((kp, pstm_f, upair_f, pss_f))
                            if len(stage) == 2 or kp == KC // 2 - 1:
                                for kp2, pstm2, upair2, pss2 in (
                                    list(stage) if kp == KC // 2 - 1 else stage[:1]
                                ):
                                    nc.scalar.activation(
                                        upair2[:, :, :width],
                                        pss2[:, :, :width], AF.Exp,
                                    )
                                    nc.vector.copy_predicated(
                                        upair2[:, :, :width],
                                        pstm2[:, :, :width].bitcast(I16),
                                        ones_bf[:, :, :width],
                                    )
                                    for j in range(2):
                                        kc = kp2 * 2 + j
                                        nc.tensor.matmul(
                                            po[:],
                                            v1[:, kc, :],
                                            upair2[:, j, :width],
                                            start=(kc == 0),
                                            stop=(kc == KC - 1),
                                        )
                                    stage.remove((kp2, pstm2, upair2, pss2))

                        numz_f = p_nz.tile([DH + 1, 512], F32, tag="nz")
                        numz = numz_f[:, :width]
                        if bi % 2 == 0:
                            nc.vector.tensor_copy(numz[:], po[:])
                        else:
                            nc.scalar.copy(numz[:], po[:])
                        o_sbn_f = p_o.tile([128, 4, DH], F32, tag="osb")
                        o_sbn = o_sbn_f[:, :nmc, :]
                        pt2 = ps_t2.tile([128, 4, DH + 1], F32, name="pt2")
                        for i in range(nmc):
                            nc.tensor.transpose(
                                pt2[:, i, :],
                                numz[:, i * 128 : (i + 1) * 128],
                                ident[0 : DH + 1, 0 : DH + 1],
                            )
                        r_cols = p_o.tile([128, 4], F32, tag="rcol")
                        nc.vector.reciprocal(
                            r_cols[:, 0:nmc], pt2[:, 0:nmc, DH]
                        )
                        for i in range(nmc):
                            if i % 2 == 0:
                                nc.vector.tensor_scalar_mul(
                                    o_sbn[:, i, :], pt2[:, i, 0:DH],
                                    r_cols[:, i : i + 1],
                                )
                            else:
                                nc.scalar.activation(
                                    o_sbn[:, i, :], pt2[:, i, 0:DH], AF.Copy,
                                    scale=r_cols[:, i : i + 1],
                                )
                        out_eng = nc.sync if bi == len(BLOCKS) - 1 else nc.gpsimd
                        out_eng.dma_start(
                            o_out[q0 : q0 + width, :].rearrange(
                                "(i p) d -> p i d", p=128
                            ),
                            o_sbn[:],
                        )

    nc.finalize()
    return nc


_CACHED_NC = None


def _get_nc():
    global _CACHED_NC
    if _CACHED_NC is None:
        _CACHED_NC = build(bacc.Bacc())
    return _CACHED_NC


def kernel(x, mask, W, b, _trace=False, _tmpdir=None):
    """Full inputs in, full output out. Shards batch across 8 neuron cores."""
    x = np.ascontiguousarray(x, dtype=np.float32)
    mask = np.ascontiguousarray(mask, dtype=np.int32)
    W = np.ascontiguousarray(W, dtype=np.float32)
    b = np.ascontiguousarray(b, dtype=np.float32)
    assert x.shape == (B, S, E) and mask.shape == (B, S, S)

    nc = _get_nc()
    in_maps = [
        {"x": x[c], "mask": mask[c], "W": W, "b": b} for c in range(N_CORES)
    ]
    res = run_bass_kernel_spmd(
        nc, in_maps, list(range(N_CORES)), trace=_trace, tmpdir=_tmpdir
    )
    out = np.stack([res.results[c]["out"] for c in range(N_CORES)])
    if _trace:
        return out, res
    return out


# revision 55
# speedup vs baseline: 1.0041x; 1.0041x over previous
"""Trainium2 Bass kernel for nn_AttentionHead (B=8, S=2048, E=1024, Dh=64).

Sharding: data-parallel over batch B across the 8 NeuronCores (one batch
element per core); W/b replicated; results gathered on host.

Per-core computation in transposed orientation (scores^T[k,q]):
  qkv = x @ W + b   (f32r; 1/sqrt(Dh) folded into W_q,b_q; the k-bias is
                     dropped entirely - it only adds a per-query constant
                     to scores, which softmax over k is invariant to up to
                     the masked entries, ~1e-3 error)
  scores^T = kT.T @ qT                  (PE f32r, 2-bank PSUM pair)
  u = exp(scores^T)                     (ACT, one instr per k-chunk pair,
                                         PSUM -> SBUF bf16)
  u = where(mask^T, 1, u)               (DVE copy_predicated; mask^T lives
                                         only in PSUM via bf16-bitcast PE
                                         transposes, re-viewed as int16)
  [num^T; Z] = [v | 1]^T @ u            (PE bf16 accumulate over k-chunks)
  out = (num * (1/Z))^T                 (numz copy, PE transpose, DVE recip,
                                         DVE/ACT per-partition scale)

DMA: transfer time occupies the issuing engine queue in the cost model
(~332 B/ns per queue) but different queues overlap. Bulk streams ride the
two compute-free queues:
  SP (sync):    x "a" half-blocks (block 0 quartered for an early PE
                start), then even mask tiles, last block's output store
  Pool (SWDGE): x "b" half-blocks, then odd mask tiles, output stores
  ACT:          only the small early W/bias loads
With DMA parallelized the kernel is PE-bound: ~66 us PE busy out of a
~79 us span (x-transposes f32r 10.2, qkv 13.7, scores 13.7,
mask-transposes 13.7, av 13.7; contraction=64 halves the array for
scores/av - unavoidable at Dh=64). DVE ~61 (copy_predicated 40 is
DVE-only and pinned), ACT ~60 (exp 41 pinned). A ~1.3 us dummy-matmul
warmup ramps the PE p-state before the first x tiles land. fp8/DoubleRow
was tried for qkv and av and REJECTED: per-element e4m3 quantization noise
(~4-6%) survives dot products at full relative strength (measured 9e-2
rel err vs the 2e-2 gate).

Cost-model span ~79.1 us/core (baseline 85.6); max rel err vs fp32
reference ~5.5e-3 (bf16 u/v rounding, dropped k-bias; f32r elsewhere).
"""

import os
import sys

sys.path.insert(0, "/opt/trn_rl_repo")

import numpy as np

import concourse.bass as bass
import concourse.tile as tile
from concourse import bacc, mybir
from concourse.masks import make_identity
from concourse.bass_utils import run_bass_kernel_spmd

F32 = mybir.dt.float32
F32R = mybir.dt.float32r
BF16 = mybir.dt.bfloat16
I32 = mybir.dt.int32
I16 = mybir.dt.int16

B, S, E, DH = 8, 2048, 1024, 64
N_CORES = 8
SCALE = 1.0 / 8.0  # 1/sqrt(DH)

MM_DT = F32R

EC = E // 128     # 8   e-chunks
SC = S // 128     # 16  s-chunks (k-chunks)
QB = S // 512     # 4   x s-blocks
KC = SC

BLOCKS = [(0, 512), (512, 512), (1024, 512), (1536, 256), (1792, 256)]

AF = mybir.ActivationFunctionType
OP = mybir.AluOpType

# bulk DMA queues: SP (no compute) and Pool (nearly idle)
MASK_ENG = ["sync", "gpsimd"]


def build(nc: bass.Bass):
    x_in = nc.dram_tensor("x", [S, E], F32, kind="ExternalInput")
    m_in = nc.dram_tensor("mask", [S, S], I32, kind="ExternalInput")
    w_in = nc.dram_tensor("W", [E, 3 * DH], F32, kind="ExternalInput")
    b_in = nc.dram_tensor("b", [3 * DH], F32, kind="ExternalInput")
    o_out = nc.dram_tensor("out", [S, DH], F32, kind="ExternalOutput")

    trace_sim = bool(os.environ.get("TRN_TRACE_SIM"))
    with tile.TileContext(nc, trace_sim=trace_sim) as tc:
        with (
            tc.tile_pool(name="persist", bufs=1) as persist,
            tc.tile_pool(name="small", bufs=1) as small,
        ):
            # ---- constants / weights (Pool queue) ------------------------
            ident = persist.tile([128, 128], F32)
            make_identity(nc, ident)
            ident_bf = persist.tile([128, 128], BF16)
            nc.vector.tensor_copy(ident_bf[:], ident[:])
            ident_r = persist.tile([128, 128], F32R)
            nc.vector.tensor_copy(ident_r[:], ident[:])
            ones_bf = persist.tile([128, 2, 512], BF16)
            nc.gpsimd.memset(ones_bf[:], 1.0)

            # PE clock warmup: ~3us of dummy matmuls from t~0.2 so the PE
            # p-state is fully ramped when the first real transposes arrive
            warm_w = small.tile([128, 128], BF16)
            nc.vector.memset(warm_w[:], 0.0)
            with tc.tile_pool(name="ps_w", bufs=1, space="PSUM") as ps_w:
                ps_warm = ps_w.tile([128, 128], F32, name="psw")
                for _ in range(12):
                    nc.tensor.matmul(
                        ps_warm[:], warm_w[:], warm_w[:], start=True, stop=True
                    )

            w_raw = small.tile([128, EC, 3 * DH], F32)
            nc.scalar.dma_start(w_raw[:], w_in.rearrange("(o p) d -> p o d", p=128))
            # stacked f32r stationaries: wst1 = [W_q*scale | W_k],
            # wst2 = [W_k | W_v]  (fp8 would be ~5-9% error: per-element
            # quantization noise survives the contraction at full relative
            # strength)
            wst1 = persist.tile([128, EC, 128], F32R)
            wst2 = persist.tile([128, EC, 128], F32R)
            nc.vector.tensor_scalar_mul(wst1[:, :, 0:64], w_raw[:, :, 0:DH], SCALE)
            nc.scalar.copy(wst1[:, :, 64:128], w_raw[:, :, DH : 2 * DH])
            nc.vector.tensor_copy(wst2[:, :, 0:64], w_raw[:, :, DH : 2 * DH])
            nc.scalar.copy(wst2[:, :, 64:128], w_raw[:, :, 2 * DH : 3 * DH])

            # b_k is dropped: a k-bias adds a per-query constant to scores,
            # which softmax over k is invariant to (the bq.k0 cross-term is
            # kept via b_q). kT is then a pure psum copy.
            b_q_raw = small.tile([64, 1], F32)
            nc.scalar.dma_start(b_q_raw[:], b_in[0:64].unsqueeze(-1))
            b_q = persist.tile([64, 1], F32)
            nc.vector.tensor_scalar_mul(b_q[:], b_q_raw[:], SCALE)
            b_v128 = persist.tile([128, 1], F32)  # v bias on lanes 64-127
            nc.scalar.dma_start(b_v128[64:128, :], b_in[128:192].unsqueeze(-1))

            # warm the ACT exp table early
            warm = small.tile([1, 1], F32)
            nc.vector.memset(warm[:], 0.0)
            warm_o = small.tile([1, 1], BF16)
            nc.scalar.activation(warm_o[:], warm[:], AF.Exp)

            # persistent activations: qv rows 0-63 = q^T, rows 64-127 = v^T
            qv = persist.tile([128, S], MM_DT)
            kT = persist.tile([64, S], MM_DT)
            v1 = persist.tile([128, SC, DH + 1], BF16)  # v natural + ones col
            nc.gpsimd.memset(v1[:, :, DH : DH + 1], 1.0)

            from contextlib import ExitStack

            mask_ctx = ExitStack()
            # 9 rotating slots: tile n+9 reuses tile n's space, whose consumer
            # block finishes well before tile n+9 is needed (no deadlock: the
            # attend consumes tiles strictly in order)
            p_m = mask_ctx.enter_context(tc.tile_pool(name="mstage", bufs=9))

            # ---- phase 1: x -> x^T -> q/k/v ------------------------------
            with (
                tc.tile_pool(name="xnat", bufs=2) as p_xnat,
                tc.tile_pool(name="xT", bufs=2) as p_xT,
                tc.tile_pool(name="ps_t", bufs=6, space="PSUM") as ps_t,
                tc.tile_pool(name="ps_mm", bufs=2, space="PSUM") as ps_mm,
            ):
                x_nats = []
                for nt in range(QB):
                    # x half-blocks: "a" half on SP, "b" half on Pool; the
                    # first block is quartered so transposes start early
                    x_nat4 = p_xnat.tile([128, 4, E], F32R)
                    if nt == 0:
                        for q, eng in enumerate((nc.sync, nc.gpsimd) * 2):
                            eng.dma_start(
                                x_nat4[:, q : q + 1, :],
                                x_in[q * 128 : (q + 1) * 128, :]
                                .rearrange("(c p) e -> p c e", p=128)
                                .bitcast(F32R),
                            )
                    else:
                        for h, eng in ((0, nc.sync), (1, nc.gpsimd)):
                            eng.dma_start(
                                x_nat4[:, h * 2 : (h + 1) * 2, :],
                                x_in[nt * 512 + h * 256 : nt * 512 + (h + 1) * 256, :]
                                .rearrange("(c p) e -> p c e", p=128)
                                .bitcast(F32R),
                            )
                    x_nats.append(x_nat4)

                # mask tile DMAs, round-robin over the three queues, emitted
                # after x so each queue drains x first (program order)
                m_tiles = {}
                for mc in range(SC):
                    m_i32 = p_m.tile([128, S], I32)
                    eng = getattr(nc, MASK_ENG[mc % 2])
                    eng.dma_start(m_i32[:], m_in[mc * 128 : (mc + 1) * 128, :])
                    m_tiles[mc] = m_i32

                for nt in range(QB):
                    x_nat4 = x_nats[nt]
                    x_T = p_xT.tile([128, EC, 512], F32R)
                    # all transposes+copies of the block first, qkv after:
                    # keeps the in-order PE queue free of qkv instructions
                    # that would stall waiting on the copies
                    for c4 in range(4):
                        for g in range(2):
                            pst = ps_t.tile([128, 512], F32R, name="pst")
                            for j4 in range(4):
                                j = g * 4 + j4
                                nc.tensor.transpose(
                                    pst[:, j4 * 128 : (j4 + 1) * 128],
                                    x_nat4[:, c4, j * 128 : (j + 1) * 128],
                                    ident_r[:],
                                )
                            dst = x_T[
                                :, g * 4 : (g + 1) * 4,
                                c4 * 128 : (c4 + 1) * 128,
                            ]
                            src = pst[:].rearrange("p (j f) -> p j f", j=4)
                            if (c4 + g) % 2 == 0:
                                nc.vector.tensor_copy(dst, src)
                            else:
                                nc.scalar.copy(dst, src)  # ACT

                    for p2 in range(2):
                        ps1 = ps_mm.tile([128, 256], F32, name="psmm")
                        ps2 = ps_mm.tile([128, 256], F32, name="psmm")
                        cs = slice(p2 * 256, (p2 + 1) * 256)
                        gsl = slice(nt * 512 + p2 * 256, nt * 512 + (p2 + 1) * 256)
                        for j in range(EC):
                            nc.tensor.matmul(
                                ps1[:], wst1[:, j, :], x_T[:, j, cs],
                                start=(j == 0), stop=(j == EC - 1),
                            )
                        for j in range(EC):
                            nc.tensor.matmul(
                                ps2[:], wst2[:, j, :], x_T[:, j, cs],
                                start=(j == 0), stop=(j == EC - 1),
                            )
                        # kT is a pure copy (k-bias dropped: softmax-invariant)
                        nc.vector.tensor_scalar_add(
                            qv[0:64, gsl], ps1[0:64, :], b_q[:]
                        )
                        nc.scalar.copy(kT[:, gsl], ps2[0:64, :])
                        nc.vector.tensor_scalar_add(
                            qv[64:128, gsl], ps2[64:128, :], b_v128[64:128, :]
                        )

                    # v natural (+ones col) for this block's four s-chunks
                    psv = ps_mm.tile([128, 256], F32R, name="psmm")
                    for j in range(4):
                        c = nt * 4 + j
                        nc.tensor.transpose(
                            psv[:, j * 64 : (j + 1) * 64],
                            qv[64:128, c * 128 : (c + 1) * 128],
                            ident_r[64:128, 64:128],
                        )
                    nc.scalar.copy(
                        v1[:, nt * 4 : (nt + 1) * 4, 0:DH],
                        psv[:, 0:256].rearrange("p (j f) -> p j f", j=4),
                    )

            # ---- phase 2: attention --------------------------------------
            with mask_ctx:
                with (
                    tc.tile_pool(name="u", bufs=6) as p_u,
                    tc.tile_pool(name="nz", bufs=3) as p_nz,
                    tc.tile_pool(name="osb", bufs=3) as p_o,
                    tc.tile_pool(name="ps_s", bufs=2, space="PSUM") as ps_s,
                    tc.tile_pool(name="ps_tm", bufs=2, space="PSUM") as ps_tm,
                    tc.tile_pool(name="ps_o", bufs=1, space="PSUM") as ps_o,
                    tc.tile_pool(name="ps_t2", bufs=1, space="PSUM") as ps_t2,
                ):
                    for bi, (q0, width) in enumerate(BLOCKS):
                        nmc = width // 128
                        mviews = [
                            m_tiles[q0 // 128 + mc][:].bitcast(BF16)
                            for mc in range(nmc)
                        ]
                        po_f = ps_o.tile([DH + 1, 512], F32, name="po")
                        po = po_f[:, :width]
                        # software-pipelined: transposes+scores of pair p
                        # are emitted before exp/cp/av of pair p-1
                        stage = []
                        for kp in range(KC // 2):
                            pstm_f = ps_tm.tile([128, 2, 512], BF16, name="pstm")
                            upair_f = p_u.tile([128, 2, 512], BF16, tag="u")
                            pss_f = ps_s.tile([128, 2, 512], F32, name="pss")
                            for j in range(2):
                                kc = kp * 2 + j
                                pstm = pstm_f[:, j, :width]
                                for mc in range(nmc):
                                    nc.tensor.matmul(
                                        pstm[:, mc * 128 : (mc + 1) * 128],
                                        mviews[mc][:, 2 * kc * 128 : 2 * (kc + 1) * 128 : 2],
                                        ident_bf[:],
                                        is_transpose=True,
                                    )
                                nc.tensor.matmul(
                                    pss_f[:, j, :width],
                                    kT[:, kc * 128 : (kc + 1) * 128],
                                    qv[0:64, q0 : q0 + width],
                                    start=True,
                                    stop=True,
                                )
                            stage.append((kp, pstm_f, upair_f, pss_f))
                            if len(stage) == 2 or kp == KC // 2 - 1:
                                for kp2, pstm2, upair2, pss2 in (
                                    list(stage) if kp == KC // 2 - 1 else stage[:1]
                                ):
                                    nc.scalar.activation(
                                        upair2[:, :, :width],
                                        pss2[:, :, :width], AF.Exp,
                                    )
                                    nc.vector.copy_predicated(
                                        upair2[:, :, :width],
                                        pstm2[:, :, :width].bitcast(I16),
                                        ones_bf[:, :, :width],
                                    )
                                    for j in range(2):
                                        kc = kp2 * 2 + j
                                        nc.tensor.matmul(
                                            po[:],
                                            v1[:, kc, :],
                                            upair2[:, j, :width],
                                            start=(kc == 0),
                                            stop=(kc == KC - 1),
                                        )
                                    stage.remove((kp2, pstm2, upair2, pss2))

                        numz_f = p_nz.tile([DH + 1, 512], F32, tag="nz")
                        numz = numz_f[:, :width]
                        if bi % 2 == 0:
                            nc.vector.tensor_copy(numz[:], po[:])
                        else:
                            nc.scalar.copy(numz[:], po[:])
                        o_sbn_f = p_o.tile([128, 4, DH], F32, tag="osb")
                        o_sbn = o_sbn_f[:, :nmc, :]
                        pt2 = ps_t2.tile([128, 4, DH + 1], F32, name="pt2")
                        for i in range(nmc):
                            nc.tensor.transpose(
                                pt2[:, i, :],
                                numz[:, i * 128 : (i + 1) * 128],
                                ident[0 : DH + 1, 0 : DH + 1],
                            )
                        r_cols = p_o.tile([128, 4], F32, tag="rcol")
                        nc.vector.reciprocal(
                            r_cols[:, 0:nmc], pt2[:, 0:nmc, DH]
                        )
                        for i in range(nmc):
                            if i % 2 == 0:
                                nc.vector.tensor_scalar_mul(
                                    o_sbn[:, i, :], pt2[:, i, 0:DH],
                                    r_cols[:, i : i + 1],
                                )
                            else:
                                nc.scalar.activation(
                                    o_sbn[:, i, :], pt2[:, i, 0:DH], AF.Copy,
                                    scale=r_cols[:, i : i + 1],
                                )
                        out_eng = nc.sync if bi == len(BLOCKS) - 1 else nc.gpsimd
                        out_eng.dma_start(
                            o_out[q0 : q0 + width, :].rearrange(
                                "(i p) d -> p i d", p=128
                            ),
                            o_sbn[:],
                        )

    nc.finalize()
    return nc


_CACHED_NC = None


def _get_nc():
    global _CACHED_NC
    if _CACHED_NC is None:
        _CACHED_NC = build(bacc.Bacc())
    return _CACHED_NC


def kernel(x, mask, W, b, _trace=False, _tmpdir=None):
    """Full inputs in, full output out. Shards batch across 8 neuron cores."""
    x = np.ascontiguousarray(x, dtype=np.float32)
    mask = np.ascontiguousarray(mask, dtype=np.int32)
    W = np.ascontiguousarray(W, dtype=np.float32)
    b = np.ascontiguousarray(b, dtype=np.float32)
    assert x.shape == (B, S, E) and mask.shape == (B, S, S)

    nc = _get_nc()
    in_maps = [
        {"x": x[c], "mask": mask[c], "W": W, "b": b} for c in range(N_CORES)
    ]
    res = run_bass_kernel_spmd(
        nc, in_maps, list(range(N_CORES)), trace=_trace, tmpdir=_tmpdir
    )
    out = np.stack([res.results[c]["out"] for c in range(N_CORES)])
    if _trace:
        return out, res
    return out
